# revision 6
# baseline (speedup 1.0000x reference)
"""Block-sparse MoE (dense expert-parallel) Trainium2 kernel.

Problem: nn_BlockSparseMoE_15882789061249
  T=1024 tokens, H=2048 hidden, F=1408 intermediate, E=16 experts, top_k=6.

Strategy (8 NeuronCores, SPMD single program):
  - Expert parallel: core c owns experts {2c, 2c+1}. wv1/w2 sharded by
    expert on the host; x and the gate are replicated (x is 8 MB vs 554 MB
    of weights, so replicating x beats an all-to-all token dispatch at this
    scale).
  - Host permutes the gate columns per core so that each core's own two
    experts land in route columns 0 and 1 -> a single SPMD program works
    for every core (top-k mask / renormalization are permutation-invariant).
  - On-core: fp32 router (logits -> exp -> top-6 via DVE max8/match_replace
    -> renormalized weights), bf16 expert matmuls (weights pre-cast and
    pre-tiled on host), SiLU on ScalarE, per-token combine via per-partition
    scalar multiply, DMA-accumulate of the two local experts into a DRAM
    partial, then an 8-core ReduceScatter; each core emits its 128-token
    output shard and the host concatenates shards.
"""

import numpy as np

T, H, F, E = 1024, 2048, 1408, 16
NCORES = 8
TOPK = 6

_CACHE = {}


def build_moe_nc(t, h, f, e, n_cores, topk=6):
    """Build + compile the SPMD Bass program for one core (same for all)."""
    import concourse.bacc as bacc
    import concourse.mybir as mybir
    import concourse.tile as tile

    f32 = mybir.dt.float32
    bf16 = mybir.dt.bfloat16
    AF = mybir.ActivationFunctionType
    Alu = mybir.AluOpType
    X = mybir.AxisListType.X

    epc = e // n_cores          # experts per core
    kh = h // 128               # contraction tiles over hidden
    kf = f // 128               # contraction tiles over intermediate
    mt = t // 128               # token tiles
    mf2 = 2 * f // 128          # fused gate+up row tiles
    tsh = t // n_cores          # output shard tokens
    nt = [(i, min(512, t - i)) for i in range(0, t, 512)]
    nh = [(i, min(512, h - i)) for i in range(0, h, 512)]

    nc = bacc.Bacc("TRN2", target_bir_lowering=False, debug=False,
                   num_devices=n_cores)

    xT = nc.dram_tensor("xT", [h, t], f32, kind="ExternalInput")
    gwT = nc.dram_tensor("gwT", [h, e], f32, kind="ExternalInput")
    wv1t = nc.dram_tensor("wv1t", [epc, kh, mf2, 128, 128], bf16,
                          kind="ExternalInput")
    w2t = nc.dram_tensor("w2t", [epc, kf, 128, h], bf16, kind="ExternalInput")
    ident = nc.dram_tensor("ident", [128, 128], f32, kind="ExternalInput")
    out_sh = nc.dram_tensor("out_shard", [tsh, h], f32, kind="ExternalOutput")

    partial = nc.dram_tensor("partial", [t, h], f32)
    rs_out = nc.dram_tensor("rs_out", [tsh, h], f32)

    with tile.TileContext(nc) as tc:
        with tc.tile_pool(name="persist", bufs=1) as pp:
            xb = pp.tile([128, kh * t], bf16, tag="xb")
            gw = pp.tile([128, kh * e], f32, tag="gw")
            ids = pp.tile([128, 128], f32, tag="ids")
            route = pp.tile([128, mt * e], f32, tag="route")
            act = pp.tile([128, epc * kf * t], bf16, tag="act")
            lg = pp.tile([128, t], f32, tag="lg")

            nc.sync.dma_start(out=ids[:], in_=ident[:, :])
            for k in range(kh):
                nc.sync.dma_start(out=gw[:, k * e:(k + 1) * e],
                                  in_=gwT[k * 128:(k + 1) * 128, :])

            # ---- load x (fp32), cast to bf16, router logits [e, t] ----
            with (tc.tile_pool(name="xload", bufs=3) as pxl,
                  tc.tile_pool(name="psr", bufs=1, space="PSUM") as ppr):
                psl = ppr.tile([128, t], f32, tag="psl")
                for k in range(kh):
                    xf = pxl.tile([128, t], f32, tag="xf")
                    nc.sync.dma_start(out=xf[:],
                                      in_=xT[k * 128:(k + 1) * 128, :])
                    nc.vector.tensor_copy(out=xb[:, k * t:(k + 1) * t],
                                          in_=xf[:])
                    for (n0, nsz) in nt:
                        nc.tensor.matmul(
                            psl[:e, n0:n0 + nsz],
                            lhsT=gw[:, k * e:(k + 1) * e],
                            rhs=xf[:, n0:n0 + nsz],
                            start=(k == 0), stop=(k == kh - 1))
                nc.vector.tensor_copy(out=lg[:e, :], in_=psl[:e, :])

            # ---- router: per token tile, top-k renormalized weights ----
            with (tc.tile_pool(name="rt", bufs=2) as prt,
                  tc.tile_pool(name="pst", bufs=2, space="PSUM") as ppt):
                for tt in range(mt):
                    ptile = ppt.tile([128, e], f32, tag="ltr")
                    nc.tensor.transpose(ptile[:, :],
                                        lg[:e, tt * 128:(tt + 1) * 128],
                                        ids[:e, :e])
                    mx = prt.tile([128, 1], f32, tag="mx")
                    nc.vector.reduce_max(out=mx[:], in_=ptile[:, :], axis=X)
                    nm = prt.tile([128, 1], f32, tag="nm")
                    nc.vector.tensor_scalar_mul(nm[:], mx[:], -1.0)
                    ev = prt.tile([128, e], f32, tag="ev")
                    nc.scalar.activation(ev[:], ptile[:, :], AF.Exp,
                                         bias=nm[:], scale=1.0)
                    t8 = prt.tile([128, 8], f32, tag="t8")
                    nc.vector.max(out=t8[:], in_=ev[:])
                    if topk < 8:
                        nc.vector.memset(t8[:, topk:], 0.0)
                    zap = prt.tile([128, e], f32, tag="zap")
                    nc.vector.match_replace(out=zap[:], in_to_replace=t8[:],
                                            in_values=ev[:], imm_value=0.0)
                    msk = prt.tile([128, e], f32, tag="msk")
                    nc.vector.tensor_sub(msk[:], ev[:], zap[:])
                    dn = prt.tile([128, 1], f32, tag="dn")
                    nc.vector.reduce_sum(out=dn[:], in_=msk[:], axis=X)
                    iv = prt.tile([128, 1], f32, tag="iv")
                    nc.vector.reciprocal(iv[:], dn[:])
                    nc.vector.tensor_scalar_mul(
                        route[:, tt * e:(tt + 1) * e], msk[:], iv[:])

            # ---- phase A: act[f, t] = silu(g) * u per local expert ----
            with (tc.tile_pool(name="wv", bufs=4) as pwv,
                  tc.tile_pool(name="sg", bufs=2) as psg,
                  tc.tile_pool(name="psa", bufs=2, space="PSUM") as ppa):
                for le in range(epc):
                    for m in range(kf):
                        pg = ppa.tile([128, t], f32, tag="pg")
                        pu = ppa.tile([128, t], f32, tag="pu")
                        for k in range(kh):
                            wg = pwv.tile([128, 128], bf16, tag="wg")
                            nc.sync.dma_start(out=wg[:], in_=wv1t[le, k, m])
                            wu = pwv.tile([128, 128], bf16, tag="wu")
                            nc.sync.dma_start(out=wu[:],
                                              in_=wv1t[le, k, m + kf])
                            # one weight load serves both N-halves
                            for (n0, nsz) in nt:
                                rh = xb[:, k * t + n0:k * t + n0 + nsz]
                                nc.tensor.matmul(pg[:, n0:n0 + nsz],
                                                 lhsT=wg[:], rhs=rh,
                                                 start=(k == 0),
                                                 stop=(k == kh - 1))
                            for (n0, nsz) in nt:
                                rh = xb[:, k * t + n0:k * t + n0 + nsz]
                                nc.tensor.matmul(pu[:, n0:n0 + nsz],
                                                 lhsT=wu[:], rhs=rh,
                                                 start=(k == 0),
                                                 stop=(k == kh - 1))
                        sgm = psg.tile([128, t], bf16, tag="sgm")
                        nc.scalar.activation(sgm[:], pg[:], AF.Sigmoid)
                        sg = psg.tile([128, t], bf16, tag="sg")
                        nc.vector.tensor_mul(out=sg[:], in0=sgm[:], in1=pg[:])
                        ai = (le * kf + m) * t
                        nc.vector.tensor_mul(out=act[:, ai:ai + t],
                                             in0=sg[:], in1=pu[:])

            # ---- phase B: y = act @ w2T, combine with route weights ----
            with (tc.tile_pool(name="w2p", bufs=kf + 3) as pw2,
                  tc.tile_pool(name="sc", bufs=3) as psc,
                  tc.tile_pool(name="psb", bufs=2, space="PSUM") as ppb):
                for le in range(epc):
                    w2ks = []
                    for k in range(kf):
                        w2k = pw2.tile([128, h], bf16, tag="w2k")
                        nc.sync.dma_start(out=w2k[:], in_=w2t[le, k])
                        w2ks.append(w2k)
                    for tt in range(mt):
                        py = ppb.tile([128, h], f32, tag="py")
                        for k in range(kf):
                            ai = (le * kf + k) * t + tt * 128
                            for (n0, nsz) in nh:
                                nc.tensor.matmul(
                                    py[:, n0:n0 + nsz],
                                    lhsT=act[:, ai:ai + 128],
                                    rhs=w2ks[k][:, n0:n0 + nsz],
                                    start=(k == 0), stop=(k == kf - 1))
                        rcol = route[:, tt * e + le:tt * e + le + 1]
                        sc = psc.tile([128, h], f32, tag="sc")
                        nc.vector.tensor_scalar_mul(sc[:], py[:, :], rcol)
                        dst = partial[tt * 128:(tt + 1) * 128, :]
                        if le == 0:
                            nc.sync.dma_start(out=dst, in_=sc[:])
                        else:
                            nc.gpsimd.dma_start(out=dst, in_=sc[:],
                                                accum_op=Alu.add)

            # ---- cross-core reduce-scatter + shard output ----
            nc.gpsimd.collective_compute(
                "ReduceScatter", Alu.add,
                replica_groups=[list(range(n_cores))],
                ins=[partial.ap().opt()],
                outs=[rs_out.ap().opt()],
            )
            nc.sync.dma_start(out=out_sh[:, :], in_=rs_out[:, :])

    nc.compile()
    return nc


def prep_inputs(x, gate_w, wv1, w2, t, h, f, e, n_cores):
    """Host-side shard/cast/tile. Returns per-core input maps."""
    import ml_dtypes
    bf16 = ml_dtypes.bfloat16

    epc = e // n_cores
    kh = h // 128
    kf = f // 128
    mf2 = 2 * f // 128

    xT = np.ascontiguousarray(x.T).astype(np.float32)        # [h, t]
    ident = np.eye(128, dtype=np.float32)

    in_maps = []
    for c in range(n_cores):
        own = list(range(c * epc, (c + 1) * epc))
        rest = [i for i in range(e) if i not in own]
        perm = own + rest
        gwT = np.ascontiguousarray(gate_w[perm].T).astype(np.float32)

        wl = wv1[own]                                        # [epc, 2f, h]
        # wv1t[le, k, m, hp, fp] = wv1[own[le], m*128+fp, k*128+hp]
        wv1tc = np.ascontiguousarray(
            wl.transpose(0, 2, 1)                            # [epc, h, 2f]
              .reshape(epc, kh, 128, mf2, 128)
              .transpose(0, 1, 3, 2, 4)).astype(bf16)

        w2l = w2[own]                                        # [epc, h, f]
        # w2t[le, k, fp, hh] = w2[own[le], hh, k*128+fp]
        w2tc = np.ascontiguousarray(
            w2l.transpose(0, 2, 1)                           # [epc, f, h]
               .reshape(epc, kf, 128, h)).astype(bf16)

        in_maps.append({
            "xT": xT,
            "gwT": gwT,
            "wv1t": wv1tc,
            "w2t": w2tc,
            "ident": ident,
        })
    return in_maps


def kernel(x, gate_w, wv1, w2, top_k):
    from concourse.bass_utils import run_bass_kernel_spmd

    assert int(top_k) == TOPK
    x = np.asarray(x, dtype=np.float32)
    gate_w = np.asarray(gate_w, dtype=np.float32)
    wv1 = np.asarray(wv1, dtype=np.float32)
    w2 = np.asarray(w2, dtype=np.float32)

    key = (T, H, F, E, NCORES)
    if key not in _CACHE:
        _CACHE[key] = build_moe_nc(T, H, F, E, NCORES, TOPK)
    nc = _CACHE[key]

    in_maps = prep_inputs(x, gate_w, wv1, w2, T, H, F, E, NCORES)
    res = run_bass_kernel_spmd(nc, in_maps, list(range(NCORES)))
    shards = [res.results[c]["out_shard"] for c in range(NCORES)]
    return np.concatenate(shards, axis=0).astype(np.float32)
